# revision 39
# baseline (speedup 1.0000x reference)
"""Trainium2 Bass kernel for a NonLocalBlock (B=2, C=256, H=W=64).

Math (reference):
    theta/phi/g = 1x1 conv of inp (C -> CH=128), L = CH*H = 8192
    attn = softmax(th @ ph^T) over kv, with th, ph (L, W=64)
    o = attn @ gg -> out = conv1x1(o) + b_out + inp

Sharding: 8 cores = 2 samples x 4 h-blocks (16 h rows each). Each core
computes the attention output rows q=(ch, h) for its h-block; the
per-core x is column-permuted so the core's own 16 h rows come first
(SPMD-identical kernel; softmax over kv is h-order invariant).

Per-core layouts (kv order = (h', ch'), q order = (h, ch)):
    Qt  (64=w,  2048=q)    phT (64=w, 8192=kv)    vaug (128=ch', 64*65)
    S^T tile t = phT[:, t*128:(t+1)*128].T @ Qt   -> psum (128=kv, q)
    E = exp(S^T); O'^T += E.T @ [V_t | 1] -> psum (65, q); row 64 = denom

exp is split between TWO engines: ACT computes exact exp; the DVE
computes a Schraudolph bit-trick exp for ~24 of the 128 kv tiles:
    int16(round(S * 128*log2e + (127*128 - 7.25)))  bitcast->  bf16
(one tensor_scalar mult+add; the fp32->int16 write conversion does the
round, the int16 bits ARE the bf16 2^(S*log2e) with linear-interp
mantissa, error factor centered to within +-3%; softmax row
normalization averages it out: measured end-to-end rel err ~4e-3 vs
the 2e-2 gate).  The offload unbinds the ACT exp chain (~1.07us/tile)
from the PE floor (S 1024 + PV 1024 cols/it = ~1.0us @2.4GHz), and the
alternating consumers decouple the S-buffer PSUM ping-pong.  The DVE
share is kept SMALL on purpose: saturating PE+ACT+DVE together trips
the chip power governor (P0), which drops the PE clock 2.4->2.0 GHz
(or a 13/16-duty dither state) and costs more than the offload wins.

dtypes: Q/K/V fp16, ACT-path E bf16, DVE-path E int16-as-bf16, matmul
accumulation fp32 in PSUM, logits fp32, out conv fp32r.

Prologue: critical x/w chunks split across the sync and scalar HW DMA
queues (gpsimd's queue has ~10us completion latency - only late cargo
rides it; each scalar-queue doorbell costs the ACT sequencer ~0.7us so
most chunks ride sync); hp0-7 conv + S(0..7) run K=64 with exp scale 2
so nothing waits on the [w;w] dup DMAs; the transpose identity is
loaded pre-cast to bf16 (a mid-stream CAST on the DVE previously
serialized the whole prologue behind the slow idt DMA).  The HAM clock
gate un-throttles the PE only after a long window of sustained
activity and its phase is free-running, so the first ~10-25us of the
stream run at half clock run-to-run; warmup matmuls keep the window
alive through the DMA waits.

Drain: the last exp is split 2x512 so the final PV retires early, and
the pass-1 out-conv runs in 128-col chunks as each transposed block
normalizes, with stores alternating the sync/scalar queues.
"""

import numpy as np

B, C, H, W = 2, 256, 64, 64
CH = C // 2          # 128
HS = H // 4          # 16 h rows per core
LQ = CH * HS         # 2048 q rows per core
NKV = 64             # kv tiles of 128 (kv = (h', ch'))
QP = 1024            # q per attention pass (PSUM budget)
PHB = 2048           # pht column base inside the merged qph tile

LOG2E = 1.4426950408889634
EXP_A = 128.0 * LOG2E            # Schraudolph scale (bf16 target)
EXP_B = 127.0 * 128.0 - 7.25     # magic + centering delta

_cached = {}


def _dve_tile(it):
    # which kv-tile exps run on the DVE (Schraudolph) vs ACT (exact).
    # Keep the DVE share SMALL: the chip's power governor (P0) drops the
    # PE clock 2.4->2.0 GHz under sustained all-engine load, and the PE
    # paces the stream -- the offload only needs to be big enough that
    # the ACT exp chain stays off the critical path (~1.07us/tile vs
    # the ~1.0us/it PE floor). it=63 breaks the ACT streak at the pass
    # boundary (three back-to-back ACT tiles stall the PSUM ping-pong).
    if it < 8 or it >= 127:
        return False
    if it < NKV:
        return it % 8 == 5 or it == 63
    if it < 112:
        return it % 4 == 1
    # strict alternation at the tail: ACT streaks let the PE idle once
    # the pass-0 drain work runs out, and the HAM gate re-throttles
    return it % 2 == 1


def _build_nc():
    import concourse.mybir as mybir
    import concourse.tile as tile
    from concourse import bacc

    f32 = mybir.dt.float32
    f16 = mybir.dt.float16
    bf16 = mybir.dt.bfloat16
    i16 = mybir.dt.int16
    AF = mybir.ActivationFunctionType
    ALU = mybir.AluOpType

    nc = bacc.Bacc("TRN2", target_bir_lowering=False, debug=False, num_devices=8)

    x0 = nc.dram_tensor("x0", [128, 4096], f16, kind="ExternalInput")
    x1 = nc.dram_tensor("x1", [128, 4096], f16, kind="ExternalInput")
    wc = nc.dram_tensor("wc", [128, 2, 256], f16, kind="ExternalInput")
    wg = nc.dram_tensor("wg", [128, 2, 128], f16, kind="ExternalInput")
    wo = nc.dram_tensor("wo", [128, 2, 128], bf16, kind="ExternalInput")
    bias2 = nc.dram_tensor("bias2", [64, 256], f32, kind="ExternalInput")
    bg = nc.dram_tensor("bg", [128, 1], f32, kind="ExternalInput")
    bo = nc.dram_tensor("bo", [128, 2], f32, kind="ExternalInput")
    identb = nc.dram_tensor("identb", [65, 65], bf16, kind="ExternalInput")
    y = nc.dram_tensor("y", [2, 128, 1024], f32, kind="ExternalOutput")

    with tile.TileContext(nc) as tc:
        with (
            tc.tile_pool(name="const", bufs=1) as cp,
            tc.tile_pool(name="big", bufs=1) as bp,
            tc.tile_pool(name="work", bufs=3) as wkp,
            tc.tile_pool(name="psum", bufs=1, space="PSUM") as pp,
        ):
            x0t = bp.tile([128, 4096], f16, tag="x0")
            x1t = bp.tile([128, 4096], f16, tag="x1")
            wct = cp.tile([128, 2, 256], f16, tag="wc")
            wgt = cp.tile([128, 2, 128], f16, tag="wg")
            wot = cp.tile([128, 2, 128], bf16, tag="wo")
            b2t = cp.tile([64, 256], f32, tag="bias2")
            bgt = cp.tile([128, 1], f32, tag="bg")
            bot = cp.tile([128, 2], f32, tag="bo")
            idtb = cp.tile([65, 65], bf16, tag="identb")
            warm16 = cp.tile([128, 512], f16, tag="warm16")

            # ---- DMA wave 1: critical path.  Only SP/ACT/gpsimd can
            # initiate DMAs; SP+ACT carry the x first chunks and the
            # conv weights; gpsimd (slow completion) carries the bias
            # rows and all late-needed cargo.
            nc.sync.dma_start(out=x0t[:, 0:256], in_=x0[:, 0:256])
            nc.scalar.dma_start(out=x1t[:, 0:256], in_=x1[:, 0:256])
            nc.sync.dma_start(out=wct[:, 0, :], in_=wc[:, 0, :])
            nc.scalar.dma_start(out=wct[:, 1, :], in_=wc[:, 1, :])
            nc.gpsimd.memset(warm16[:], 0.125)
            nc.gpsimd.dma_start(out=b2t[:], in_=bias2[:])
            nc.gpsimd.dma_start(out=bgt[:], in_=bg[:])
            nc.sync.dma_start(out=x0t[:, 256:512], in_=x0[:, 256:512])
            nc.scalar.dma_start(out=x1t[:, 256:512], in_=x1[:, 256:512])
            nc.scalar.dma_start(out=wgt[:], in_=wg[:])
            nc.gpsimd.dma_start(out=idtb[:], in_=identb[:])
            # second chunks: [512:1024] lands early enough for hp4-7 in
            # the prologue; [1024:2048] covers hp8-15 consumed from it=4
            nc.sync.dma_start(out=x0t[:, 512:1024], in_=x0[:, 512:1024])
            nc.scalar.dma_start(out=x1t[:, 512:1024], in_=x1[:, 512:1024])
            nc.sync.dma_start(out=x0t[:, 1024:2048], in_=x0[:, 1024:2048])
            nc.scalar.dma_start(out=x1t[:, 1024:2048], in_=x1[:, 1024:2048])

            def emit_late_dmas(step):
                # x second halves in 1024-col chunks so the in-loop pht
                # dup DMAs (emitted between steps) don't queue behind a
                # single 512KB transfer
                if step == 0:
                    nc.sync.dma_start(out=x0t[:, 2048:3072],
                                      in_=x0[:, 2048:3072])
                elif step == 1:
                    nc.scalar.dma_start(out=x1t[:, 2048:3072],
                                        in_=x1[:, 2048:3072])
                elif step == 2:
                    nc.sync.dma_start(out=x0t[:, 3072:4096],
                                      in_=x0[:, 3072:4096])
                elif step == 3:
                    nc.scalar.dma_start(out=x1t[:, 3072:4096],
                                        in_=x1[:, 3072:4096])
                elif step == 4:
                    nc.scalar.dma_start(out=wot[:], in_=wo[:])
                    nc.scalar.dma_start(out=bot[:], in_=bo[:])

            # preload the exp table set while DMAs run
            warm = wkp.tile([1, 1], f32, tag="warm")
            nc.scalar.activation(warm[:], warm16[0:1, 0:1], AF.Exp)

            for dmy in range(4):
                dps = pp.tile([128, 512], f32, tag="ot", name=f"dmy{dmy}")
                nc.tensor.matmul(dps[:], lhsT=warm16[:, 0:128],
                                 rhs=warm16[:], start=True, stop=True)

            # [w; w] duplicated along partitions so the S matmul runs at
            # K=128 (HAM never un-throttles the PE clock for K=64; phi
            # half of wc/bias2 pre-scaled 0.5 on host so the dup sums
            # to the original dot product). qt and pht are views of ONE
            # tile so a single strided copy writes both 128-col blocks.
            qph = bp.tile([128, PHB + 8192], f16, tag="qph")
            qt = qph[:, 0:PHB]                             # (w2, q)
            pht = qph[:, PHB:PHB + 8192]                   # (w2, kv)
            vaug = bp.tile([128, NKV * 65], bf16, tag="vaug")
            osb = bp.tile([128, 1024], bf16, tag="osb")    # o (ch, (h, w))
            otsb = bp.tile([65, LQ], bf16, tag="otsb")     # O'^T staged
            ysb0 = bp.tile([128, 1024], f32, tag="ysb0")
            ysb1 = bp.tile([128, 1024], f32, tag="ysb1")
            xsb0 = bp.tile([128, 1024], f32, tag="xsb0")
            xsb1 = bp.tile([128, 1024], f32, tag="xsb1")
            vaug3 = vaug.rearrange("p (t j) -> p t j", j=65)
            nc.vector.memset(vaug3[:, :, 64:65], 1.0)
            qphb = qph.rearrange("p (a c) -> p a c", c=128)

            b2v = b2t.rearrange("p (a c) -> p a c", c=128)

            def emit_hp_pro(hp):
                # prologue h-pairs (theta+phi): 2 matmuls, then per half
                # one DVE add applying [bth|bph/2], writing the qt and
                # pht 128-col blocks of the merged tile in one strided op
                ps = pp.tile([128, 256], f32, tag="conv", bufs=2,
                             name=f"c{hp}")
                nc.tensor.matmul(ps[:],
                                 lhsT=x0t[:, hp * 128:(hp + 1) * 128],
                                 rhs=wct[:, 0, :], start=True, stop=False)
                nc.tensor.matmul(ps[:],
                                 lhsT=x1t[:, hp * 128:(hp + 1) * 128],
                                 rhs=wct[:, 1, :], start=False, stop=True)
                ps3 = ps.rearrange("p (a c) -> p a c", c=128)
                for hh in range(2):
                    h = 2 * hp + hh
                    nc.vector.tensor_tensor(
                        out=qphb[0:64, h:h + 17:16, :],
                        in0=ps3[hh * 64:(hh + 1) * 64, :, :],
                        in1=b2v[:], op=ALU.add)
                if hp == 3:
                    # dup q block 0 + kv tiles 0-7 (consumed from it=8;
                    # S(0..7) run K=64 so they don't wait on these).
                    # SP is otherwise idle here, so both ride sync.
                    nc.sync.dma_start(out=qph[64:128, 0:1024],
                                      in_=qph[0:64, 0:1024])
                    nc.sync.dma_start(out=qph[64:128, PHB:PHB + 1024],
                                      in_=qph[0:64, PHB:PHB + 1024])
                if hp == 7:
                    # dup kv tiles 8-15 + q block 1 (pass 1).  The ACT
                    # doorbell hides in the exp(0)->exp(1) wait window.
                    nc.scalar.dma_start(out=pht[64:128, 1024:2048],
                                        in_=pht[0:64, 1024:2048])
                    nc.sync.dma_start(out=qt[64:128, 1024:2048],
                                      in_=qt[0:64, 1024:2048])

            def emit_hp_x0(hp, ps):
                # steady-state h-pairs (phi only, hp>=8) split over two
                # iterations: x0 half here, x1 half + adds next
                nc.tensor.matmul(ps[:, 0:128],
                                 lhsT=x0t[:, hp * 128:(hp + 1) * 128],
                                 rhs=wct[:, 0, 128:256],
                                 start=True, stop=False)

            def emit_hp(hp, ps):
                nc.tensor.matmul(ps[:, 0:128],
                                 lhsT=x1t[:, hp * 128:(hp + 1) * 128],
                                 rhs=wct[:, 1, 128:256],
                                 start=False, stop=True)
                for hh in range(2):
                    h = 2 * hp + hh
                    nc.vector.tensor_tensor(
                        out=pht[0:64, h * 128:(h + 1) * 128],
                        in0=ps[hh * 64:(hh + 1) * 64, 0:128],
                        in1=b2t[:, 128:256], op=ALU.add)
                if hp % 2 == 1 and hp > 8:
                    c0 = (hp - 1) * 256
                    eng = nc.sync if hp % 4 == 1 else nc.scalar
                    eng.dma_start(out=pht[64:128, c0:c0 + 512],
                                  in_=pht[0:64, c0:c0 + 512])

            def emit_g(n):
                # g conv chunk -> vaug (values only; ones pre-memset)
                ps = pp.tile([128, 512], f32, tag="conv", bufs=2,
                             name=f"g{n}")
                nc.tensor.matmul(ps[:], lhsT=wgt[:, 0, :],
                                 rhs=x0t[:, n * 512:(n + 1) * 512],
                                 start=True, stop=False)
                nc.tensor.matmul(ps[:], lhsT=wgt[:, 1, :],
                                 rhs=x1t[:, n * 512:(n + 1) * 512],
                                 start=False, stop=True)
                nc.vector.tensor_scalar(
                    out=vaug3[:, n * 8:(n + 1) * 8, 0:64],
                    in0=ps.rearrange("p (t j) -> p t j", j=64)[:],
                    scalar1=bgt[:, 0:1], scalar2=None, op0=ALU.add)

            def emit_lh(lh, norm_eng):
                # transpose + normalize one 128-q block of O'^T
                trp = pp.tile([128, 65], bf16, tag="conv", bufs=2,
                              name=f"tr{lh}")
                nc.tensor.transpose(trp[:],
                                    otsb[:, lh * 128:(lh + 1) * 128],
                                    idtb[:])
                rden = wkp.tile([128, 1], f32, tag="rden", name=f"rd{lh}")
                nc.vector.reciprocal(rden[:], trp[:, 64:65])
                if norm_eng == "act":
                    nc.scalar.activation(osb[:, lh * 64:(lh + 1) * 64],
                                         trp[:, 0:64], AF.Copy,
                                         scale=rden[:])
                else:
                    nc.vector.tensor_scalar(
                        out=osb[:, lh * 64:(lh + 1) * 64],
                        in0=trp[:, 0:64],
                        scalar1=rden[:], scalar2=None, op0=ALU.mult)

            def emit_xsb(m):
                # precombine residual + out-conv bias while attention runs
                xsb = xsb0 if m == 0 else xsb1
                xres = x0t if m == 0 else x1t
                # residual re-read from the fp16 x tile (own h-block is
                # cols 0:1024): saves a 1MB fp32 load; |x|<6 so the fp16
                # quantization (~2e-4 of output scale) is negligible
                nc.vector.tensor_scalar(out=xsb[:], in0=xres[:, 0:1024],
                                        scalar1=bot[:, m:m + 1], scalar2=None,
                                        op0=ALU.add)

            def emit_y(m, c0, c1, store_eng=None):
                # out conv for columns [c0:c1] + (bias+residual) + store
                ysb = ysb0 if m == 0 else ysb1
                xsb = xsb0 if m == 0 else xsb1
                yp = pp.tile([128, 512], f32, tag="conv", bufs=2,
                             name=f"yp{m}{c0}")
                nc.tensor.matmul(yp[:, 0:c1 - c0], lhsT=wot[:, m, :],
                                 rhs=osb[:, c0:c1], start=True, stop=True)
                nc.vector.tensor_tensor(
                    out=ysb[:, c0:c1], in0=yp[:, 0:c1 - c0],
                    in1=xsb[:, c0:c1], op=ALU.add)
                eng = store_eng if store_eng is not None else nc.sync
                eng.dma_start(out=y[m, :, c0:c1], in_=ysb[:, c0:c1])

            # ---- attention: software-pipelined over 2 q passes of 1024 ---
            NIT = 2 * NKV
            ets = {}
            otps = {}

            def emit_s(it):
                # first eight kv tiles run K=64 from the single-copy rows
                # (exp scale 2 compensates the pre-halved phi)
                p, t = it // NKV, it % NKV
                k = 64 if it < 8 else 128
                sp = pp.tile([128, QP], f32, tag="s", bufs=2, name=f"sp{it}")
                for c in range(2):
                    nc.tensor.matmul(
                        sp[:, c * 512:(c + 1) * 512],
                        lhsT=pht[0:k, t * 128:(t + 1) * 128],
                        rhs=qt[0:k, p * QP + c * 512: p * QP + (c + 1) * 512],
                        start=True, stop=True)
                if _dve_tile(it):
                    eti = wkp.tile([128, QP], i16, tag="ei", bufs=4,
                                   name=f"ei{it}")
                    nc.vector.tensor_scalar(
                        out=eti[:], in0=sp[:],
                        scalar1=float(EXP_A), scalar2=float(EXP_B),
                        op0=ALU.mult, op1=ALU.add)
                    ets[it] = (eti, True)
                    return
                et = wkp.tile([128, QP], bf16, tag="e", bufs=5, name=f"et{it}")
                sc = 2.0 if it < 8 else 1.0
                if it == 0 or it == NIT - 1:
                    # split the first exp (its first half only needs the
                    # hp0/hp1 q columns, so ACT starts ~1.5us earlier) and
                    # the last one (the final PV/drain chain starts early)
                    nc.scalar.activation(et[:, 0:512], sp[:, 0:512],
                                         AF.Exp, scale=sc)
                    nc.scalar.activation(et[:, 512:1024], sp[:, 512:1024],
                                         AF.Exp, scale=sc)
                else:
                    nc.scalar.activation(et[:], sp[:], AF.Exp, scale=sc)
                ets[it] = (et, False)

            def emit_pv(it):
                p, t = it // NKV, it % NKV
                if t == 0:
                    otps[p] = pp.tile([65, QP], f32, tag="ot", bufs=1,
                                      name=f"otp{p}")
                otp = otps[p]
                et, is_dve = ets.pop(it)
                last = t == NKV - 1
                for c in range(2):
                    rhs = et[:, c * 512:(c + 1) * 512]
                    if is_dve:
                        rhs = rhs.bitcast(mybir.dt.bfloat16)
                    nc.tensor.matmul(
                        otp[:, c * 512:(c + 1) * 512],
                        lhsT=vaug3[:, t, :],
                        rhs=rhs,
                        start=(t == 0), stop=last,
                        skip_group_check=True)
                    if last:
                        # stage each finished half immediately; final
                        # pass puts half on ACT (idle once exp is done)
                        if p == 1 and c == 1:
                            nc.scalar.activation(
                                otsb[:, p * QP + c * 512:p * QP + (c + 1) * 512],
                                otp[:, c * 512:(c + 1) * 512], AF.Copy)
                        else:
                            nc.vector.tensor_copy(
                                otsb[:, p * QP + c * 512:p * QP + (c + 1) * 512],
                                otp[:, c * 512:(c + 1) * 512])
                if last and p == 0:
                    for lh in range(8):
                        todo.append(lambda lh=lh: emit_lh(lh, "vec"))
                    todo.append(lambda: emit_xsb(0))
                    todo.append(lambda: emit_xsb(1))
                    todo.append(lambda: emit_y(0, 0, 512))
                    todo.append(lambda: emit_y(1, 0, 512))

            from collections import deque
            todo = deque()

            # ---- prologue: hp0-7 conv + first S tiles, overlapped with
            # the x DMA waves (warmups keep the HAM duty high meanwhile)
            for hp in range(4):
                emit_hp_pro(hp)
            emit_s(0)
            emit_g(0)
            for hp in range(4, 8):
                emit_hp_pro(hp)
            emit_s(1)

            n_hp = 8
            n_g = 1
            hp_pair = None
            for it in range(2, NIT):
                emit_s(it)
                emit_pv(it - 2)
                if it == 3:
                    emit_late_dmas(0)
                elif it == 5:
                    emit_late_dmas(1)
                elif it == 9:
                    emit_late_dmas(2)
                elif it == 11:
                    emit_late_dmas(3)
                elif it == 13:
                    emit_late_dmas(4)
                if it >= 4 and n_hp < 32:
                    # hp(n) completes by it=2(n-8)+5, consumed at it=2n.
                    # Two hp tiles pack into one (128,512) pool buffer.
                    if it % 2 == 0:
                        if n_hp % 2 == 0:
                            hp_pair = pp.tile([128, 512], f32, tag="conv",
                                              bufs=2, name=f"cp{n_hp}")
                        off = (n_hp % 2) * 256
                        hp_ps = hp_pair[:, off:off + 256]
                        emit_hp_x0(n_hp, hp_ps)
                    else:
                        emit_hp(n_hp, hp_ps)
                        n_hp += 1
                if it % 5 == 3 and n_g < 8:
                    emit_g(n_g)
                    n_g += 1
                if todo and it % 2 == 1 and it >= 69:
                    todo.popleft()()
            emit_pv(NIT - 2)
            emit_pv(NIT - 1)
            while todo:
                todo.popleft()()
            for lh in range(8, 16):
                emit_lh(lh, "act" if lh % 2 == 0 else "vec")
                c0 = lh * 64
                emit_y(0, c0, c0 + 64, store_eng=nc.scalar)
                emit_y(1, c0, c0 + 64, store_eng=nc.sync)

    nc.compile()
    return nc


def _get_nc():
    if "nc" not in _cached:
        _cached["nc"] = _build_nc()
    return _cached["nc"]


LAST_EXEC_NS = None
LAST_TRACE_DIR = None


def kernel(inp, w_theta, b_theta, w_phi, b_phi, w_g, b_g, w_out, b_out):
    import os
    from concourse.bass_utils import run_bass_kernel_spmd

    nc = _get_nc()

    f = np.float32
    c = np.ascontiguousarray
    import ml_dtypes

    # [w_theta | w_phi] concatenated, as (c_lo, half, 256) fp16
    wcat = np.concatenate([w_theta.T, w_phi.T * 0.5], axis=1).astype(f)
    wc3 = c(wcat.reshape(2, 128, 256).transpose(1, 0, 2).astype(np.float16))
    wg3 = c(w_g.T.reshape(2, 128, CH).transpose(1, 0, 2).astype(np.float16))
    wo3 = c(w_out.reshape(2, 128, CH).transpose(2, 0, 1)
            .astype(ml_dtypes.bfloat16))  # [ch, m, co]
    brow = np.concatenate([b_theta, b_phi * 0.5])
    bias2v = c(np.tile(brow.astype(f), (64, 1)))
    bg1 = c(b_g.astype(f)[:, None])
    bo2 = c(b_out.reshape(2, 128).T.astype(f))
    identb = c(np.eye(65, dtype=ml_dtypes.bfloat16))

    in_maps = []
    for core in range(8):
        b, k = core // 4, core % 4
        x = inp[b].reshape(C, H, W).astype(f)
        # own h-block first, then the rest: kernel is h-order agnostic
        perm = list(range(HS * k, HS * (k + 1))) + \
            [h for h in range(H) if not (HS * k <= h < HS * (k + 1))]
        xp = x[:, perm, :].reshape(C, H * W)
        xp16 = xp.astype(np.float16)
        in_maps.append({
            "x0": c(xp16[:128]), "x1": c(xp16[128:]),
            "wc": wc3, "wg": wg3, "wo": wo3,
            "bias2": bias2v, "bg": bg1, "bo": bo2, "identb": identb,
        })

    trace = bool(os.environ.get("NLB_TRACE"))
    tmpdir = os.environ.get("NLB_TRACE_DIR") or None
    res = run_bass_kernel_spmd(nc, in_maps, list(range(8)), trace=trace,
                               tmpdir=tmpdir)
    global LAST_EXEC_NS, LAST_TRACE_DIR
    LAST_EXEC_NS = res.exec_time_ns
    LAST_TRACE_DIR = tmpdir

    out = np.empty((B, C, H, W), dtype=f)
    for core in range(8):
        b, k = core // 4, core % 4
        yc = res.results[core]["y"].reshape(C, HS, W)
        out[b, :, HS * k:HS * (k + 1), :] = yc
    return out


# revision 41
# speedup vs baseline: 1.0173x; 1.0173x over previous
"""Trainium2 Bass kernel for a NonLocalBlock (B=2, C=256, H=W=64).

Math (reference):
    theta/phi/g = 1x1 conv of inp (C -> CH=128), L = CH*H = 8192
    attn = softmax(th @ ph^T) over kv, with th, ph (L, W=64)
    o = attn @ gg -> out = conv1x1(o) + b_out + inp

Sharding: 8 cores = 2 samples x 4 h-blocks (16 h rows each). Each core
computes the attention output rows q=(ch, h) for its h-block; the
per-core x is column-permuted so the core's own 16 h rows come first
(SPMD-identical kernel; softmax over kv is h-order invariant).

Per-core layouts (kv order = (h', ch'), q order = (h, ch)):
    Qt  (64=w,  2048=q)    phT (64=w, 8192=kv)    vaug (128=ch', 64*65)
    S^T tile t = phT[:, t*128:(t+1)*128].T @ Qt   -> psum (128=kv, q)
    E = exp(S^T); O'^T += E.T @ [V_t | 1] -> psum (65, q); row 64 = denom

exp is split between TWO engines: ACT computes exact exp; the DVE
computes a Schraudolph bit-trick exp for ~24 of the 128 kv tiles:
    int16(round(S * 128*log2e + (127*128 - 7.25)))  bitcast->  bf16
(one tensor_scalar mult+add; the fp32->int16 write conversion does the
round, the int16 bits ARE the bf16 2^(S*log2e) with linear-interp
mantissa, error factor centered to within +-3%; softmax row
normalization averages it out: measured end-to-end rel err ~4e-3 vs
the 2e-2 gate).  The offload unbinds the ACT exp chain (~1.07us/tile)
from the PE floor (S 1024 + PV 1024 cols/it = ~1.0us @2.4GHz), and the
alternating consumers decouple the S-buffer PSUM ping-pong.  The DVE
share is kept SMALL on purpose: saturating PE+ACT+DVE together trips
the chip power governor (P0), which drops the PE clock 2.4->2.0 GHz
(or a 13/16-duty dither state) and costs more than the offload wins.

dtypes: Q/K/V fp16, ACT-path E bf16, DVE-path E int16-as-bf16, matmul
accumulation fp32 in PSUM, logits fp32, out conv fp32r.

Prologue: critical x/w chunks split across the sync and scalar HW DMA
queues (gpsimd's queue has ~10us completion latency - only late cargo
rides it; each scalar-queue doorbell costs the ACT sequencer ~0.7us so
most chunks ride sync); hp0-7 conv + S(0..7) run K=64 with exp scale 2
so nothing waits on the [w;w] dup DMAs; the transpose identity is
loaded pre-cast to bf16 (a mid-stream CAST on the DVE previously
serialized the whole prologue behind the slow idt DMA).  The HAM clock
gate un-throttles the PE only after a long window of sustained
activity and its phase is free-running, so the first ~10-25us of the
stream run at half clock run-to-run; warmup matmuls keep the window
alive through the DMA waits.

Drain: the last exp is split 2x512 so the final PV retires early, and
the pass-1 out-conv runs in 128-col chunks as each transposed block
normalizes, with stores alternating the sync/scalar queues.
"""

import numpy as np

B, C, H, W = 2, 256, 64, 64
CH = C // 2          # 128
HS = H // 4          # 16 h rows per core
LQ = CH * HS         # 2048 q rows per core
NKV = 64             # kv tiles of 128 (kv = (h', ch'))
QP = 1024            # q per attention pass (PSUM budget)
PHB = 2048           # pht column base inside the merged qph tile

LOG2E = 1.4426950408889634
EXP_A = 128.0 * LOG2E            # Schraudolph scale (bf16 target)
EXP_B = 127.0 * 128.0 - 7.25     # magic + centering delta

_cached = {}


def _dve_tile(it):
    # which kv-tile exps run on the DVE (Schraudolph) vs ACT (exact).
    # Keep the DVE share SMALL: the chip's power governor (P0) drops the
    # PE clock 2.4->2.0 GHz under sustained all-engine load, and the PE
    # paces the stream -- the offload only needs to be big enough that
    # the ACT exp chain stays off the critical path (~1.07us/tile vs
    # the ~1.0us/it PE floor). it=63 breaks the ACT streak at the pass
    # boundary (three back-to-back ACT tiles stall the PSUM ping-pong).
    if it < 8 or it >= 127:
        return False
    if it < NKV:
        return it % 8 == 5 or it == 63
    if it < 112:
        return it % 3 == 1
    # strict alternation at the tail: ACT streaks let the PE idle once
    # the pass-0 drain work runs out, and the HAM gate re-throttles
    return it % 2 == 1


def _build_nc():
    import concourse.mybir as mybir
    import concourse.tile as tile
    from concourse import bacc

    f32 = mybir.dt.float32
    f16 = mybir.dt.float16
    bf16 = mybir.dt.bfloat16
    i16 = mybir.dt.int16
    AF = mybir.ActivationFunctionType
    ALU = mybir.AluOpType

    nc = bacc.Bacc("TRN2", target_bir_lowering=False, debug=False, num_devices=8)

    x0 = nc.dram_tensor("x0", [128, 4096], f16, kind="ExternalInput")
    x1 = nc.dram_tensor("x1", [128, 4096], f16, kind="ExternalInput")
    xs0 = nc.dram_tensor("xs0", [128, 1024], f32, kind="ExternalInput")
    xs1 = nc.dram_tensor("xs1", [128, 1024], f32, kind="ExternalInput")
    wc = nc.dram_tensor("wc", [128, 2, 256], f16, kind="ExternalInput")
    wg = nc.dram_tensor("wg", [128, 2, 128], f16, kind="ExternalInput")
    wo = nc.dram_tensor("wo", [128, 2, 128], bf16, kind="ExternalInput")
    bias2 = nc.dram_tensor("bias2", [64, 256], f32, kind="ExternalInput")
    bg = nc.dram_tensor("bg", [128, 1], f32, kind="ExternalInput")
    bo = nc.dram_tensor("bo", [128, 2], f32, kind="ExternalInput")
    identb = nc.dram_tensor("identb", [65, 65], bf16, kind="ExternalInput")
    y = nc.dram_tensor("y", [2, 128, 1024], f32, kind="ExternalOutput")

    with tile.TileContext(nc) as tc:
        with (
            tc.tile_pool(name="const", bufs=1) as cp,
            tc.tile_pool(name="big", bufs=1) as bp,
            tc.tile_pool(name="work", bufs=3) as wkp,
            tc.tile_pool(name="psum", bufs=1, space="PSUM") as pp,
        ):
            x0t = bp.tile([128, 4096], f16, tag="x0")
            x1t = bp.tile([128, 4096], f16, tag="x1")
            xs0t = bp.tile([128, 1024], f32, tag="xs0")
            xs1t = bp.tile([128, 1024], f32, tag="xs1")
            wct = cp.tile([128, 2, 256], f16, tag="wc")
            wgt = cp.tile([128, 2, 128], f16, tag="wg")
            wot = cp.tile([128, 2, 128], bf16, tag="wo")
            b2t = cp.tile([64, 256], f32, tag="bias2")
            bgt = cp.tile([128, 1], f32, tag="bg")
            bot = cp.tile([128, 2], f32, tag="bo")
            idtb = cp.tile([65, 65], bf16, tag="identb")
            warm16 = cp.tile([128, 512], f16, tag="warm16")

            # ---- DMA wave 1: critical path.  Only SP/ACT/gpsimd can
            # initiate DMAs; SP+ACT carry the x first chunks and the
            # conv weights; gpsimd (slow completion) carries the bias
            # rows and all late-needed cargo.
            nc.sync.dma_start(out=x0t[:, 0:256], in_=x0[:, 0:256])
            nc.scalar.dma_start(out=x1t[:, 0:256], in_=x1[:, 0:256])
            nc.sync.dma_start(out=wct[:, 0, :], in_=wc[:, 0, :])
            nc.scalar.dma_start(out=wct[:, 1, :], in_=wc[:, 1, :])
            nc.gpsimd.memset(warm16[:], 0.125)
            nc.gpsimd.dma_start(out=b2t[:], in_=bias2[:])
            nc.gpsimd.dma_start(out=bgt[:], in_=bg[:])
            nc.sync.dma_start(out=x0t[:, 256:512], in_=x0[:, 256:512])
            nc.scalar.dma_start(out=x1t[:, 256:512], in_=x1[:, 256:512])
            nc.scalar.dma_start(out=wgt[:], in_=wg[:])
            nc.gpsimd.dma_start(out=idtb[:], in_=identb[:])
            # second chunks: [512:1024] lands early enough for hp4-7 in
            # the prologue; [1024:2048] covers hp8-15 consumed from it=4
            nc.sync.dma_start(out=x0t[:, 512:1024], in_=x0[:, 512:1024])
            nc.scalar.dma_start(out=x1t[:, 512:1024], in_=x1[:, 512:1024])
            nc.sync.dma_start(out=x0t[:, 1024:2048], in_=x0[:, 1024:2048])
            nc.scalar.dma_start(out=x1t[:, 1024:2048], in_=x1[:, 1024:2048])
            nc.gpsimd.dma_start(out=xs0t[:], in_=xs0[:])
            nc.gpsimd.dma_start(out=xs1t[:], in_=xs1[:])

            def emit_late_dmas(step):
                # x second halves in 1024-col chunks so the in-loop pht
                # dup DMAs (emitted between steps) don't queue behind a
                # single 512KB transfer
                if step == 0:
                    nc.sync.dma_start(out=x0t[:, 2048:3072],
                                      in_=x0[:, 2048:3072])
                elif step == 1:
                    nc.scalar.dma_start(out=x1t[:, 2048:3072],
                                        in_=x1[:, 2048:3072])
                elif step == 2:
                    nc.sync.dma_start(out=x0t[:, 3072:4096],
                                      in_=x0[:, 3072:4096])
                elif step == 3:
                    nc.scalar.dma_start(out=x1t[:, 3072:4096],
                                        in_=x1[:, 3072:4096])
                elif step == 4:
                    nc.scalar.dma_start(out=wot[:], in_=wo[:])
                    nc.scalar.dma_start(out=bot[:], in_=bo[:])

            # preload the exp table set while DMAs run
            warm = wkp.tile([1, 1], f32, tag="warm")
            nc.scalar.activation(warm[:], warm16[0:1, 0:1], AF.Exp)

            for dmy in range(4):
                dps = pp.tile([128, 512], f32, tag="ot", name=f"dmy{dmy}")
                nc.tensor.matmul(dps[:], lhsT=warm16[:, 0:128],
                                 rhs=warm16[:], start=True, stop=True)

            # [w; w] duplicated along partitions so the S matmul runs at
            # K=128 (HAM never un-throttles the PE clock for K=64; phi
            # half of wc/bias2 pre-scaled 0.5 on host so the dup sums
            # to the original dot product). qt and pht are views of ONE
            # tile so a single strided copy writes both 128-col blocks.
            qph = bp.tile([128, PHB + 8192], f16, tag="qph")
            qt = qph[:, 0:PHB]                             # (w2, q)
            pht = qph[:, PHB:PHB + 8192]                   # (w2, kv)
            vaug = bp.tile([128, NKV * 65], bf16, tag="vaug")
            osb = bp.tile([128, 1024], bf16, tag="osb")    # o (ch, (h, w))
            otsb = bp.tile([65, LQ], bf16, tag="otsb")     # O'^T staged
            ysb0 = bp.tile([128, 1024], f32, tag="ysb0")
            ysb1 = bp.tile([128, 1024], f32, tag="ysb1")
            xsb0 = bp.tile([128, 1024], f32, tag="xsb0")
            xsb1 = bp.tile([128, 1024], f32, tag="xsb1")
            vaug3 = vaug.rearrange("p (t j) -> p t j", j=65)
            nc.vector.memset(vaug3[:, :, 64:65], 1.0)
            qphb = qph.rearrange("p (a c) -> p a c", c=128)

            b2v = b2t.rearrange("p (a c) -> p a c", c=128)

            def emit_hp_pro(hp):
                # prologue h-pairs (theta+phi): 2 matmuls, then per half
                # one DVE add applying [bth|bph/2], writing the qt and
                # pht 128-col blocks of the merged tile in one strided op
                ps = pp.tile([128, 256], f32, tag="conv", bufs=2,
                             name=f"c{hp}")
                nc.tensor.matmul(ps[:],
                                 lhsT=x0t[:, hp * 128:(hp + 1) * 128],
                                 rhs=wct[:, 0, :], start=True, stop=False)
                nc.tensor.matmul(ps[:],
                                 lhsT=x1t[:, hp * 128:(hp + 1) * 128],
                                 rhs=wct[:, 1, :], start=False, stop=True)
                ps3 = ps.rearrange("p (a c) -> p a c", c=128)
                for hh in range(2):
                    h = 2 * hp + hh
                    nc.vector.tensor_tensor(
                        out=qphb[0:64, h:h + 17:16, :],
                        in0=ps3[hh * 64:(hh + 1) * 64, :, :],
                        in1=b2v[:], op=ALU.add)
                if hp == 3:
                    # dup q block 0 + kv tiles 0-7 (consumed from it=8;
                    # S(0..7) run K=64 so they don't wait on these).
                    # SP is otherwise idle here, so both ride sync.
                    nc.sync.dma_start(out=qph[64:128, 0:1024],
                                      in_=qph[0:64, 0:1024])
                    nc.sync.dma_start(out=qph[64:128, PHB:PHB + 1024],
                                      in_=qph[0:64, PHB:PHB + 1024])
                if hp == 7:
                    # dup kv tiles 8-15 + q block 1 (pass 1).  The ACT
                    # doorbell hides in the exp(0)->exp(1) wait window.
                    nc.scalar.dma_start(out=pht[64:128, 1024:2048],
                                        in_=pht[0:64, 1024:2048])
                    nc.sync.dma_start(out=qt[64:128, 1024:2048],
                                      in_=qt[0:64, 1024:2048])

            def emit_hp_x0(hp, ps):
                # steady-state h-pairs (phi only, hp>=8) split over two
                # iterations: x0 half here, x1 half + adds next
                nc.tensor.matmul(ps[:, 0:128],
                                 lhsT=x0t[:, hp * 128:(hp + 1) * 128],
                                 rhs=wct[:, 0, 128:256],
                                 start=True, stop=False)

            def emit_hp(hp, ps):
                nc.tensor.matmul(ps[:, 0:128],
                                 lhsT=x1t[:, hp * 128:(hp + 1) * 128],
                                 rhs=wct[:, 1, 128:256],
                                 start=False, stop=True)
                for hh in range(2):
                    h = 2 * hp + hh
                    nc.vector.tensor_tensor(
                        out=pht[0:64, h * 128:(h + 1) * 128],
                        in0=ps[hh * 64:(hh + 1) * 64, 0:128],
                        in1=b2t[:, 128:256], op=ALU.add)
                if hp % 2 == 1 and hp > 8:
                    c0 = (hp - 1) * 256
                    eng = nc.sync if hp % 4 == 1 else nc.scalar
                    eng.dma_start(out=pht[64:128, c0:c0 + 512],
                                  in_=pht[0:64, c0:c0 + 512])

            def emit_g(n):
                # g conv chunk -> vaug (values only; ones pre-memset)
                ps = pp.tile([128, 512], f32, tag="conv", bufs=2,
                             name=f"g{n}")
                nc.tensor.matmul(ps[:], lhsT=wgt[:, 0, :],
                                 rhs=x0t[:, n * 512:(n + 1) * 512],
                                 start=True, stop=False)
                nc.tensor.matmul(ps[:], lhsT=wgt[:, 1, :],
                                 rhs=x1t[:, n * 512:(n + 1) * 512],
                                 start=False, stop=True)
                nc.vector.tensor_scalar(
                    out=vaug3[:, n * 8:(n + 1) * 8, 0:64],
                    in0=ps.rearrange("p (t j) -> p t j", j=64)[:],
                    scalar1=bgt[:, 0:1], scalar2=None, op0=ALU.add)

            def emit_lh(lh, norm_eng):
                # transpose + normalize one 128-q block of O'^T
                trp = pp.tile([128, 65], bf16, tag="conv", bufs=2,
                              name=f"tr{lh}")
                nc.tensor.transpose(trp[:],
                                    otsb[:, lh * 128:(lh + 1) * 128],
                                    idtb[:])
                rden = wkp.tile([128, 1], f32, tag="rden", name=f"rd{lh}")
                nc.vector.reciprocal(rden[:], trp[:, 64:65])
                if norm_eng == "act":
                    nc.scalar.activation(osb[:, lh * 64:(lh + 1) * 64],
                                         trp[:, 0:64], AF.Copy,
                                         scale=rden[:])
                else:
                    nc.vector.tensor_scalar(
                        out=osb[:, lh * 64:(lh + 1) * 64],
                        in0=trp[:, 0:64],
                        scalar1=rden[:], scalar2=None, op0=ALU.mult)

            def emit_xsb(m):
                # precombine residual + out-conv bias while attention runs
                xsb = xsb0 if m == 0 else xsb1
                xres = xs0t if m == 0 else xs1t
                nc.vector.tensor_scalar(out=xsb[:], in0=xres[:],
                                        scalar1=bot[:, m:m + 1], scalar2=None,
                                        op0=ALU.add)

            def emit_y(m, c0, c1, store_eng=None):
                # out conv for columns [c0:c1] + (bias+residual) + store
                ysb = ysb0 if m == 0 else ysb1
                xsb = xsb0 if m == 0 else xsb1
                yp = pp.tile([128, 512], f32, tag="conv", bufs=2,
                             name=f"yp{m}{c0}")
                nc.tensor.matmul(yp[:, 0:c1 - c0], lhsT=wot[:, m, :],
                                 rhs=osb[:, c0:c1], start=True, stop=True)
                nc.vector.tensor_tensor(
                    out=ysb[:, c0:c1], in0=yp[:, 0:c1 - c0],
                    in1=xsb[:, c0:c1], op=ALU.add)
                eng = store_eng if store_eng is not None else nc.sync
                eng.dma_start(out=y[m, :, c0:c1], in_=ysb[:, c0:c1])

            # ---- attention: software-pipelined over 2 q passes of 1024 ---
            NIT = 2 * NKV
            ets = {}
            otps = {}

            def emit_s(it):
                # first eight kv tiles run K=64 from the single-copy rows
                # (exp scale 2 compensates the pre-halved phi)
                p, t = it // NKV, it % NKV
                k = 64 if it < 8 else 128
                sp = pp.tile([128, QP], f32, tag="s", bufs=2, name=f"sp{it}")
                for c in range(2):
                    nc.tensor.matmul(
                        sp[:, c * 512:(c + 1) * 512],
                        lhsT=pht[0:k, t * 128:(t + 1) * 128],
                        rhs=qt[0:k, p * QP + c * 512: p * QP + (c + 1) * 512],
                        start=True, stop=True)
                if _dve_tile(it):
                    eti = wkp.tile([128, QP], i16, tag="ei", bufs=4,
                                   name=f"ei{it}")
                    nc.vector.tensor_scalar(
                        out=eti[:], in0=sp[:],
                        scalar1=float(EXP_A), scalar2=float(EXP_B),
                        op0=ALU.mult, op1=ALU.add)
                    ets[it] = (eti, True)
                    return
                et = wkp.tile([128, QP], bf16, tag="e", bufs=5, name=f"et{it}")
                sc = 2.0 if it < 8 else 1.0
                if it == 0 or it == NIT - 1:
                    # split the first exp (its first half only needs the
                    # hp0/hp1 q columns, so ACT starts ~1.5us earlier) and
                    # the last one (the final PV/drain chain starts early)
                    nc.scalar.activation(et[:, 0:512], sp[:, 0:512],
                                         AF.Exp, scale=sc)
                    nc.scalar.activation(et[:, 512:1024], sp[:, 512:1024],
                                         AF.Exp, scale=sc)
                else:
                    nc.scalar.activation(et[:], sp[:], AF.Exp, scale=sc)
                ets[it] = (et, False)

            def emit_pv(it):
                p, t = it // NKV, it % NKV
                if t == 0:
                    otps[p] = pp.tile([65, QP], f32, tag="ot", bufs=1,
                                      name=f"otp{p}")
                otp = otps[p]
                et, is_dve = ets.pop(it)
                last = t == NKV - 1
                for c in range(2):
                    rhs = et[:, c * 512:(c + 1) * 512]
                    if is_dve:
                        rhs = rhs.bitcast(mybir.dt.bfloat16)
                    nc.tensor.matmul(
                        otp[:, c * 512:(c + 1) * 512],
                        lhsT=vaug3[:, t, :],
                        rhs=rhs,
                        start=(t == 0), stop=last,
                        skip_group_check=True)
                    if last:
                        # stage each finished half immediately; final
                        # pass puts half on ACT (idle once exp is done)
                        if p == 1 and c == 1:
                            nc.scalar.activation(
                                otsb[:, p * QP + c * 512:p * QP + (c + 1) * 512],
                                otp[:, c * 512:(c + 1) * 512], AF.Copy)
                        else:
                            nc.vector.tensor_copy(
                                otsb[:, p * QP + c * 512:p * QP + (c + 1) * 512],
                                otp[:, c * 512:(c + 1) * 512])
                if last and p == 0:
                    for lh in range(8):
                        todo.append(lambda lh=lh: emit_lh(lh, "vec"))
                    todo.append(lambda: emit_xsb(0))
                    todo.append(lambda: emit_xsb(1))
                    todo.append(lambda: emit_y(0, 0, 512))
                    todo.append(lambda: emit_y(1, 0, 512))

            from collections import deque
            todo = deque()

            # ---- prologue: hp0-7 conv + first S tiles, overlapped with
            # the x DMA waves (warmups keep the HAM duty high meanwhile)
            for hp in range(4):
                emit_hp_pro(hp)
            emit_s(0)
            emit_g(0)
            for hp in range(4, 8):
                emit_hp_pro(hp)
            emit_s(1)

            n_hp = 8
            n_g = 1
            hp_pair = None
            for it in range(2, NIT):
                emit_s(it)
                emit_pv(it - 2)
                if it == 3:
                    emit_late_dmas(0)
                elif it == 5:
                    emit_late_dmas(1)
                elif it == 9:
                    emit_late_dmas(2)
                elif it == 11:
                    emit_late_dmas(3)
                elif it == 13:
                    emit_late_dmas(4)
                if it >= 4 and n_hp < 32:
                    # hp(n) completes by it=2(n-8)+5, consumed at it=2n.
                    # Two hp tiles pack into one (128,512) pool buffer.
                    if it % 2 == 0:
                        if n_hp % 2 == 0:
                            hp_pair = pp.tile([128, 512], f32, tag="conv",
                                              bufs=2, name=f"cp{n_hp}")
                        off = (n_hp % 2) * 256
                        hp_ps = hp_pair[:, off:off + 256]
                        emit_hp_x0(n_hp, hp_ps)
                    else:
                        emit_hp(n_hp, hp_ps)
                        n_hp += 1
                if it % 5 == 3 and n_g < 8:
                    emit_g(n_g)
                    n_g += 1
                if todo and it % 2 == 1 and it >= 69:
                    todo.popleft()()
            emit_pv(NIT - 2)
            emit_pv(NIT - 1)
            while todo:
                todo.popleft()()
            for lh in range(8, 16):
                emit_lh(lh, "act")
                if lh >= 9 and lh % 2 == 1 and lh < 15:
                    c0 = 512 + (lh - 9) * 64
                    emit_y(0, c0, c0 + 128, store_eng=nc.scalar)
                    emit_y(1, c0, c0 + 128, store_eng=nc.sync)
                elif lh == 15:
                    # final chunk split in two so the end-of-kernel barrier
                    # waits on a 64KB transfer instead of 128KB
                    emit_y(0, 896, 960, store_eng=nc.scalar)
                    emit_y(1, 896, 960, store_eng=nc.sync)
                    emit_y(0, 960, 1024, store_eng=nc.scalar)
                    emit_y(1, 960, 1024, store_eng=nc.sync)

    nc.compile()
    return nc


def _get_nc():
    if "nc" not in _cached:
        _cached["nc"] = _build_nc()
    return _cached["nc"]


LAST_EXEC_NS = None
LAST_TRACE_DIR = None


def kernel(inp, w_theta, b_theta, w_phi, b_phi, w_g, b_g, w_out, b_out):
    import os
    from concourse.bass_utils import run_bass_kernel_spmd

    nc = _get_nc()

    f = np.float32
    c = np.ascontiguousarray
    import ml_dtypes

    # [w_theta | w_phi] concatenated, as (c_lo, half, 256) fp16
    wcat = np.concatenate([w_theta.T, w_phi.T * 0.5], axis=1).astype(f)
    wc3 = c(wcat.reshape(2, 128, 256).transpose(1, 0, 2).astype(np.float16))
    wg3 = c(w_g.T.reshape(2, 128, CH).transpose(1, 0, 2).astype(np.float16))
    wo3 = c(w_out.reshape(2, 128, CH).transpose(2, 0, 1)
            .astype(ml_dtypes.bfloat16))  # [ch, m, co]
    brow = np.concatenate([b_theta, b_phi * 0.5])
    bias2v = c(np.tile(brow.astype(f), (64, 1)))
    bg1 = c(b_g.astype(f)[:, None])
    bo2 = c(b_out.reshape(2, 128).T.astype(f))
    identb = c(np.eye(65, dtype=ml_dtypes.bfloat16))

    in_maps = []
    for core in range(8):
        b, k = core // 4, core % 4
        x = inp[b].reshape(C, H, W).astype(f)
        # own h-block first, then the rest: kernel is h-order agnostic
        perm = list(range(HS * k, HS * (k + 1))) + \
            [h for h in range(H) if not (HS * k <= h < HS * (k + 1))]
        xp = x[:, perm, :].reshape(C, H * W)
        xp16 = xp.astype(np.float16)
        in_maps.append({
            "x0": c(xp16[:128]), "x1": c(xp16[128:]),
            "xs0": c(xp[:128, :1024]), "xs1": c(xp[128:, :1024]),
            "wc": wc3, "wg": wg3, "wo": wo3,
            "bias2": bias2v, "bg": bg1, "bo": bo2, "identb": identb,
        })

    trace = bool(os.environ.get("NLB_TRACE"))
    tmpdir = os.environ.get("NLB_TRACE_DIR") or None
    res = run_bass_kernel_spmd(nc, in_maps, list(range(8)), trace=trace,
                               tmpdir=tmpdir)
    global LAST_EXEC_NS, LAST_TRACE_DIR
    LAST_EXEC_NS = res.exec_time_ns
    LAST_TRACE_DIR = tmpdir

    out = np.empty((B, C, H, W), dtype=f)
    for core in range(8):
        b, k = core // 4, core % 4
        yc = res.results[core]["y"].reshape(C, HS, W)
        out[b, :, HS * k:HS * (k + 1), :] = yc
    return out
